# revision 12
# baseline (speedup 1.0000x reference)
"""Trainium2 Bass/Tile kernel: symmetric contrastive loss (CLIP-style).

v1 redesign vs baseline:
  * Host pre-transposes img/num features -> no PE transposes on device.
  * Row-norm factors are folded into the main-pass Exp via the per-partition
    scale AP (itl stays un-normalized); col-norm (incl. 1/temp) is folded
    into npf.
  * Every core locally computes npf column-blocks 0..NREP-1 (replicated
    inputs, SPMD-identical) so ~125us of main-pass work needs no AllGather;
    a dummy collective fired first absorbs the collective bootstrap.
  * Main pass: CW=2048 column supertiles; ACT does a single Exp per tile
    (PSUM->SBUF bf16, no accum_out); DVE scalar_tensor_tensor both
    accumulates colacc (bf16) and emits cumulative per-row sums via
    accum_out; row sums are recovered by differencing the cumulative
    values.
  * ReLU/bias/square/normalize moved off ACT onto DVE; ACT does only
    Exp/Ln (one table set).
  * Single tail AllReduce carrying [colsums | rowpart | diagsum].
"""

import numpy as np

N_CORES = 8
B = 16384
D_IMG = 2048
D_NUM = 256
P = 128
NREP_BLOCKS = 4          # npf col-blocks every core computes locally

_NC_CACHE = {}


def build(b_total=B, d_img=D_IMG, d_num=D_NUM, n_cores=N_CORES):
    key = (b_total, d_img, d_num, n_cores)
    if key in _NC_CACHE:
        return _NC_CACHE[key]

    import concourse.bacc as bacc
    import concourse.bass as bass
    import concourse.mybir as mybir
    import concourse.tile as tile

    dt = mybir.dt
    AF = mybir.ActivationFunctionType
    Alu = mybir.AluOpType
    AX = mybir.AxisListType
    f32 = dt.float32
    bf16 = dt.bfloat16

    BL = b_total // n_cores          # local rows per core
    CW = BL                          # column supertile width (= rank block)
    NCT = n_cores                    # supertiles = rank blocks
    NH = CW // 512                   # 512-wide matmul chunks per supertile
    NRC = BL // 128                  # 128-row chunks (main pass)
    NMLP = BL // 512                 # 512-row MLP chunks per block
    KI = d_img // 128
    KN = d_num // 128
    NREP = min(NREP_BLOCKS, NCT)
    ARW = b_total + 64               # AllReduce payload width
    PSW = max(CW, 2048)              # psum slot width (4 banks)

    nc = bacc.Bacc("TRN2", target_bir_lowering=False, debug=False,
                   num_devices=n_cores)

    imgT = nc.dram_tensor("imgT", [d_img, BL], f32, kind="ExternalInput").ap()
    numT = nc.dram_tensor("numT", [d_num, BL], f32, kind="ExternalInput").ap()
    numR = nc.dram_tensor("numR", [d_num, NREP * BL], f32,
                          kind="ExternalInput").ap()
    Wi1 = nc.dram_tensor("Wi1", [d_img, P], f32, kind="ExternalInput").ap()
    bi1 = nc.dram_tensor("bi1", [P, 1], f32, kind="ExternalInput").ap()
    Wi2 = nc.dram_tensor("Wi2", [P, P], f32, kind="ExternalInput").ap()
    bi2 = nc.dram_tensor("bi2", [P, 1], f32, kind="ExternalInput").ap()
    Wn1 = nc.dram_tensor("Wn1", [d_num, P], f32, kind="ExternalInput").ap()
    bn1 = nc.dram_tensor("bn1", [P, 1], f32, kind="ExternalInput").ap()
    Wn2 = nc.dram_tensor("Wn2", [P, P], f32, kind="ExternalInput").ap()
    bn2 = nc.dram_tensor("bn2", [P, 1], f32, kind="ExternalInput").ap()
    ltm = nc.dram_tensor("log_temp", [1, 1], f32, kind="ExternalInput").ap()
    loss = nc.dram_tensor("loss", [1, 1], f32, kind="ExternalOutput").ap()

    rg = [list(range(n_cores))]

    with tile.TileContext(nc) as tc:
        with (
            tc.tile_pool(name="sb", bufs=1) as sb,
            tc.tile_pool(name="st", bufs=2) as st,
            tc.tile_pool(name="vs", bufs=2) as vs,
            tc.tile_pool(name="ps", bufs=2, space="PSUM") as ps,
            tc.tile_pool(name="dram", bufs=1, space="DRAM") as dram,
        ):
            # ---------------- DRAM scratch ----------------
            dumm_in = dram.tile([1, 8], f32)
            dumm_out = dram.tile([1, 8], f32, addr_space="Shared")
            ag_in = dram.tile([P, BL], bf16)
            ag_out = dram.tile([n_cores * P, BL], bf16, addr_space="Shared")
            ar_in = dram.tile([1, ARW], f32)
            ar_out = dram.tile([1, ARW], f32, addr_space="Shared")
            vsq_i = dram.tile([1, BL], f32)     # img |z|^2 per local row
            ddot_d = dram.tile([1, BL], f32)    # raw diag dot per local row

            # ---------------- dummy collective: absorb bootstrap ----------
            zpad = sb.tile([1, 64], f32)
            nc.vector.memset(zpad[:], 0.0)
            nc.sync.dma_start(dumm_in[:], zpad[:1, :8])
            nc.gpsimd.collective_compute(
                "AllReduce", Alu.add, replica_groups=rg,
                ins=[dumm_in.opt()], outs=[dumm_out.opt()])

            # ---------------- constants / weights ----------------
            ones_kb = sb.tile([P, 1], bf16)
            nc.vector.memset(ones_kb[:], 1.0)
            ones_kf = sb.tile([P, 1], f32)
            nc.vector.memset(ones_kf[:], 1.0)
            ones_1f = sb.tile([1, P], f32)
            nc.vector.memset(ones_1f[:], 1.0)

            lt_sb = sb.tile([1, 1], f32)
            nc.sync.dma_start(lt_sb[:], ltm)
            nlt = sb.tile([1, 1], f32)          # -log_temp (num-side bias)
            nc.vector.tensor_scalar_mul(nlt[:], lt_sb[:], -1.0)

            bn1_sb = sb.tile([P, 1], f32)
            nc.sync.dma_start(bn1_sb[:], bn1)
            bn2_sb = sb.tile([P, 1], f32)
            nc.sync.dma_start(bn2_sb[:], bn2)
            bi1_sb = sb.tile([P, 1], f32)
            nc.sync.dma_start(bi1_sb[:], bi1)
            bi2_sb = sb.tile([P, 1], f32)
            nc.sync.dma_start(bi2_sb[:], bi2)

            # num weights (fp32 load + DVE cast)
            wn1_f = sb.tile([P, KN * P], f32)
            nc.sync.dma_start(wn1_f.rearrange("p (k m) -> p k m", k=KN),
                              Wn1.rearrange("(k p) m -> p k m", p=P))
            wn1_sb = sb.tile([P, KN * P], bf16)
            nc.vector.tensor_copy(wn1_sb[:], wn1_f[:])
            wn2_f = sb.tile([P, P], f32)
            nc.sync.dma_start(wn2_f[:], Wn2)
            wn2_sb = sb.tile([P, P], bf16)
            nc.vector.tensor_copy(wn2_sb[:], wn2_f[:])
            wi2_f = sb.tile([P, P], f32)
            nc.sync.dma_start(wi2_f[:], Wi2)
            wi2_sb = sb.tile([P, P], bf16)
            nc.vector.tensor_copy(wi2_sb[:], wi2_f[:])


            # ---------------- persistent SBUF ----------------
            npf = sb.tile([P, b_total], bf16)    # gathered/replicated num proj
            itl = sb.tile([P, BL], bf16)         # raw img proj (z, no norm)
            ntl = sb.tile([P, BL], bf16)         # own normalized num proj
            colacc = sb.tile([P, b_total], bf16)
            sacc = sb.tile([P, NRC * NCT], f32)  # cumulative row sums
            inv_img = sb.tile([P, NRC], f32)     # per-row 1/|z|, [row%128, rc]

            xstage = sb.tile([P, max(KN * BL, KI * P)], f32)  # fp32 staging
            xsv = xstage[:, 0:KN * BL].rearrange("p (k r) -> p k r", k=KN)

            # Wi1 via the shared staging tile (freed before num streaming)
            wi1_sb = sb.tile([P, KI * P], bf16)
            nc.sync.dma_start(
                xstage[:, 0:KI * P].rearrange("p (k m) -> p k m", k=KI),
                Wi1.rearrange("(k p) m -> p k m", p=P))
            nc.vector.tensor_copy(wi1_sb[:], xstage[:, 0:KI * P])

            def num_mlp_chunk(xk, m, outp, obase):
                """One 512-row num chunk: MLP + normalize (incl 1/temp).
                xk: bf16 view [P, KN, BL]; writes outp[:, obase+m*512:+512]."""
                sl = slice(obase + m * 512, obase + m * 512 + 512)
                slot = ps.tile([P, PSW], f32, tag="mm", name="numslot")
                for k in range(KN):
                    nc.tensor.matmul(slot[:, 0:512],
                                     wn1_sb[:, k * P:(k + 1) * P],
                                     xk[:, k, m * 512:(m + 1) * 512],
                                     start=(k == 0), stop=(k == KN - 1))
                h1 = st.tile([P, 512], bf16, tag="h1", name="h1n")
                nc.vector.tensor_scalar(h1[:], slot[:, 0:512], bn1_sb[:], 0.0,
                                        op0=Alu.add, op1=Alu.max)
                nc.tensor.matmul(slot[:, 512:1024], wn2_sb[:], h1[:])
                z = st.tile([P, 512], bf16, tag="z", name="zn")
                nc.vector.tensor_scalar(z[:], slot[:, 512:1024], bn2_sb[:],
                                        None, op0=Alu.add)
                sq = st.tile([P, 512], bf16, tag="sq", name="sqn")
                nc.vector.tensor_mul(sq[:], z[:], z[:])
                nc.tensor.matmul(slot[:1, 1024:1536], ones_kb[:], sq[:])
                lnv = vs.tile([1, 512], f32, tag="lnv", name="lnvn")
                nc.scalar.activation(lnv[:], slot[:1, 1024:1536], AF.Ln)
                inv = vs.tile([1, 512], f32, tag="inv", name="invn")
                nc.scalar.activation(inv[:], lnv[:], AF.Exp,
                                     bias=nlt[:], scale=-0.5)
                nc.tensor.matmul(slot[:, 1536:2048], ones_1f[:], inv[:])
                nc.vector.tensor_mul(outp[:, sl], z[:], slot[:, 1536:2048])

            # ---------------- num own block -> AllGather ----------------
            nc.sync.dma_start(xsv, numT.rearrange("(k p) r -> p k r", p=P))
            xn_sb = st.tile([P, KN * BL], bf16, tag="xn", name="xn_own")
            nc.vector.tensor_copy(xn_sb[:], xstage[:, 0:KN * BL])
            xnv = xn_sb.rearrange("p (k r) -> p k r", k=KN)
            for m in range(NMLP):
                num_mlp_chunk(xnv, m, ntl, 0)
            nc.sync.dma_start(ag_in[:], ntl[:])
            nc.gpsimd.collective_compute(
                "AllGather", Alu.bypass, replica_groups=rg,
                ins=[ag_in.opt()], outs=[ag_out.opt()])

            # ---------------- replicated num blocks -> npf[0:NREP] --------
            for rb in range(NREP):
                nc.sync.dma_start(
                    xsv,
                    numR.rearrange("(k p) r -> p k r", p=P)[
                        :, :, rb * BL:(rb + 1) * BL])
                xr = st.tile([P, KN * BL], bf16, tag="xn", name="xn_rep")
                nc.vector.tensor_copy(xr[:], xstage[:, 0:KN * BL])
                xrv = xr.rearrange("p (k r) -> p k r", k=KN)
                for m in range(NMLP):
                    num_mlp_chunk(xrv, m, npf, rb * BL)

            # npf blocks NREP..NCT-1 come from the AllGather
            if NREP < NCT:
                npf_v = npf.rearrange("p (r c) -> p r c", c=BL)
                nc.sync.dma_start(
                    npf_v[:, NREP:NCT, :],
                    ag_out.rearrange("(r p) n -> p r n", p=P)[:, NREP:NCT, :])

            # ---------------- img branch (streamed 512-row blocks) --------
            NB = max(1, BL // 512)
            for b in range(NB):
                xs = st.tile([P, KI, 512], bf16, tag="xsi", name="xsi")
                nc.gpsimd.dma_start(
                    xs[:],
                    imgT.rearrange("(k p) r -> p k r", p=P)[
                        :, :, b * 512:(b + 1) * 512])
                slot = ps.tile([P, PSW], f32, tag="mm", name="imgslot")
                for k in range(KI):
                    nc.tensor.matmul(slot[:, 0:512],
                                     wi1_sb[:, k * P:(k + 1) * P],
                                     xs[:, k, :],
                                     start=(k == 0), stop=(k == KI - 1))
                h1 = st.tile([P, 512], bf16, tag="h1", name="h1i")
                nc.vector.tensor_scalar(h1[:], slot[:, 0:512], bi1_sb[:], 0.0,
                                        op0=Alu.add, op1=Alu.max)
                nc.tensor.matmul(slot[:, 512:1024], wi2_sb[:], h1[:])
                sl = slice(b * 512, (b + 1) * 512)
                nc.vector.tensor_scalar(itl[:, sl], slot[:, 512:1024],
                                        bi2_sb[:], None, op0=Alu.add)
                sq = st.tile([P, 512], bf16, tag="sq", name="sqi")
                nc.vector.tensor_mul(sq[:], itl[:, sl], itl[:, sl])
                nc.tensor.matmul(slot[:1, 1024:1536], ones_kb[:], sq[:])
                # raw diag dot for these rows: sum_f z_i * ntl_i
                prod = st.tile([P, 512], bf16, tag="sq", name="prod")
                nc.vector.tensor_mul(prod[:], itl[:, sl], ntl[:, sl])
                nc.tensor.matmul(slot[:1, 1536:2048], ones_kb[:], prod[:])
                vstg = vs.tile([1, 1024], f32, tag="vstg", name="vstg")
                nc.vector.tensor_copy(vstg[:], slot[:1, 1024:2048])
                nc.sync.dma_start(vsq_i[:1, sl], vstg[:1, 0:512])
                nc.sync.dma_start(ddot_d[:1, sl], vstg[:1, 512:1024])
                # inv_img[:, 4b:4b+4] = exp(-0.5*ln v) via DRAM reshape
                nrc_b = 512 // 128
                vrs = vs.tile([P, nrc_b], f32, tag="vrs", name="vrs")
                nc.sync.dma_start(
                    vrs[:],
                    vsq_i.rearrange("o (rc p) -> (o p) rc", p=P)[
                        :, b * nrc_b:(b + 1) * nrc_b])
                lnr = vs.tile([P, nrc_b], f32, tag="lnr", name="lnr")
                nc.scalar.activation(lnr[:], vrs[:], AF.Ln)
                nc.scalar.activation(inv_img[:, b * nrc_b:(b + 1) * nrc_b],
                                     lnr[:], AF.Exp, scale=-0.5)

            # ---------------- main pass ----------------
            # phase 1: replicated cts (no AllGather dep), then phase 2.
            # Per-ct rc order must be ascending (cumulative-rowsum trick).
            def main_tile(rc, ct):
                slot = ps.tile([P, PSW], f32, tag="mm", name="plog")
                for h in range(NH):
                    nc.tensor.matmul(
                        slot[:, h * 512:(h + 1) * 512],
                        itl[:, rc * P:(rc + 1) * P],
                        npf[:, ct * CW + h * 512:ct * CW + (h + 1) * 512])
                e = st.tile([P, CW], bf16, tag="e", name="e", bufs=4)
                nc.scalar.activation(e[:], slot[:, 0:CW], AF.Exp,
                                     scale=inv_img[:, rc:rc + 1])
                cslice = colacc[:, ct * CW:(ct + 1) * CW]
                sidx = rc * NCT + ct
                if rc == 0:
                    nc.vector.tensor_scalar(
                        cslice, e[:], 1.0, None, op0=Alu.mult, op1=Alu.add,
                        accum_out=sacc[:, sidx:sidx + 1])
                else:
                    nc.vector.scalar_tensor_tensor(
                        cslice, e[:], 1.0, cslice,
                        op0=Alu.mult, op1=Alu.add,
                        accum_out=sacc[:, sidx:sidx + 1])

            for rc in range(NRC):
                for ct in range(NREP):
                    main_tile(rc, ct)
            for rc in range(NRC):
                for ct in range(NREP, NCT):
                    main_tile(rc, ct)

            # ---------------- column sums -> AllReduce payload ----------
            for ct in range(NCT):
                slot = ps.tile([P, PSW], f32, tag="mm", name="pcs")
                for h in range(NH):
                    nc.tensor.matmul(
                        slot[:1, h * 512:(h + 1) * 512], ones_kb[:],
                        colacc[:, ct * CW + h * 512:ct * CW + (h + 1) * 512])
                cstg = vs.tile([1, CW], f32, tag="cstg", name="cstg", bufs=1)
                nc.vector.tensor_copy(cstg[:], slot[:1, 0:CW])
                nc.sync.dma_start(ar_in[:1, ct * CW:(ct + 1) * CW], cstg[:])

            # ---------------- row part + diag ----------------
            T = sb.tile([P, NRC], f32)
            nc.vector.reduce_sum(
                T[:], sacc.rearrange("p (rc ct) -> p rc ct", ct=NCT),
                axis=AX.X)
            rs = sb.tile([P, NRC], f32)
            nc.vector.tensor_copy(rs[:, 0:1], T[:, 0:1])
            if NRC > 1:
                nc.vector.tensor_sub(rs[:, 1:NRC], T[:, 1:NRC],
                                     T[:, 0:NRC - 1])
            lse = sb.tile([P, NRC], f32)
            nc.scalar.activation(lse[:], rs[:], AF.Ln)
            lsum = sb.tile([P, 1], f32)
            nc.vector.reduce_sum(lsum[:], lse[:], axis=AX.X)

            dd_r = sb.tile([P, NRC], f32)
            nc.sync.dma_start(
                dd_r[:], ddot_d.rearrange("o (rc p) -> (o p) rc", p=P))
            dd = sb.tile([P, NRC], f32)
            nc.vector.tensor_mul(dd[:], dd_r[:], inv_img[:])
            dsum_p = sb.tile([P, 1], f32)
            nc.vector.reduce_sum(dsum_p[:], dd[:], axis=AX.X)

            slot = ps.tile([P, PSW], f32, tag="mm", name="scal")
            nc.tensor.matmul(slot[:1, 0:1], ones_kf[:], lsum[:])
            nc.tensor.matmul(slot[:1, 512:513], ones_kf[:], dsum_p[:])
            sstg = sb.tile([1, 2], f32)
            nc.vector.tensor_copy(sstg[:1, 0:1], slot[:1, 0:1])
            nc.vector.tensor_copy(sstg[:1, 1:2], slot[:1, 512:513])
            nc.sync.dma_start(ar_in[:1, b_total:b_total + 2], sstg[:])
            nc.sync.dma_start(ar_in[:1, b_total + 2:ARW], zpad[:1, :62])

            # ---------------- AllReduce + final ----------------
            nc.gpsimd.collective_compute(
                "AllReduce", Alu.add, replica_groups=rg,
                ins=[ar_in.opt()], outs=[ar_out.opt()])

            csb = sb.tile([P, b_total // P], f32)
            nc.sync.dma_start(
                csb[:],
                ar_out[:1, :b_total].rearrange("o (a b) -> (o a) b", a=P))
            sc2 = sb.tile([1, 2], f32)
            nc.sync.dma_start(sc2[:], ar_out[:1, b_total:b_total + 2])
            lse_c = sb.tile([P, b_total // P], f32)
            nc.scalar.activation(lse_c[:], csb[:], AF.Ln)
            csum_p = sb.tile([P, 1], f32)
            nc.vector.reduce_sum(csum_p[:], lse_c[:], axis=AX.X)
            slot2 = ps.tile([P, PSW], f32, tag="mm", name="fin")
            nc.tensor.matmul(slot2[:1, 0:1], ones_kf[:], csum_p[:])
            t1 = sb.tile([1, 1], f32)
            nc.vector.tensor_add(t1[:], slot2[:1, 0:1], sc2[:1, 0:1])
            t2 = sb.tile([1, 1], f32)
            nc.vector.scalar_tensor_tensor(
                t2[:], sc2[:1, 1:2], -2.0, t1[:], op0=Alu.mult, op1=Alu.add)
            lsb = sb.tile([1, 1], f32)
            nc.vector.tensor_scalar_mul(lsb[:], t2[:], 1.0 / (2.0 * b_total))
            nc.sync.dma_start(loss, lsb[:])

    nc.compile()
    _NC_CACHE[key] = nc
    return nc


def shard_inputs(inputs, b_total=B, n_cores=N_CORES):
    BL = b_total // n_cores
    nrep = min(NREP_BLOCKS, n_cores)
    img = np.asarray(inputs["img_feat"], dtype=np.float32)
    num = np.asarray(inputs["num_feat"], dtype=np.float32)
    imgT = np.ascontiguousarray(img.T)           # [d_img, b_total]
    numT = np.ascontiguousarray(num.T)           # [d_num, b_total]
    numR = np.ascontiguousarray(numT[:, :nrep * BL])

    def mat(name):
        return np.ascontiguousarray(np.asarray(inputs[name], dtype=np.float32))

    def col(name):
        return np.ascontiguousarray(
            np.asarray(inputs[name], dtype=np.float32).reshape(P, 1))

    lt = np.asarray(inputs["log_temp"], dtype=np.float32).reshape(1, 1)
    shared = {
        "Wi1": mat("Wi1"), "Wi2": mat("Wi2"),
        "Wn1": mat("Wn1"), "Wn2": mat("Wn2"),
        "bi1": col("bi1"), "bi2": col("bi2"),
        "bn1": col("bn1"), "bn2": col("bn2"),
        "log_temp": np.ascontiguousarray(lt),
        "numR": numR,
    }
    maps = []
    for c in range(n_cores):
        m = dict(shared)
        m["imgT"] = np.ascontiguousarray(imgT[:, c * BL:(c + 1) * BL])
        m["numT"] = np.ascontiguousarray(numT[:, c * BL:(c + 1) * BL])
        maps.append(m)
    return maps


def run(inputs, trace=False, **kw):
    from concourse.bass_utils import run_bass_kernel_spmd
    nc = build()
    res = run_bass_kernel_spmd(nc, shard_inputs(inputs),
                               core_ids=list(range(N_CORES)), trace=trace, **kw)
    val = np.asarray(res.results[0]["loss"], dtype=np.float32).reshape(())
    return val, res


def kernel(**inputs):
    val, _ = run(inputs)
    return val
